# revision 9
# baseline (speedup 1.0000x reference)
"""Trainium2 Bass kernel for nn_BoundaryDistillationLoss.

loss = mean((|grad(softmax(s))| - |grad(softmax(t))|)^2) with depthwise 3x3
Sobel gradients. Expanded as  [ sum(qs) + sum(qt) - 2*sum(sqrt(qs*qt)) ] / N
where q = gx^2 + gy^2, so no per-tensor sqrt is needed (one sqrt per pair).

Data parallel over B*H rows (2048) across 8 cores; each core gets a
(C, 258, W) halo-padded shard per tensor.  On-chip layout: h-rows on SBUF
partitions, (c, w) on the free dim.  The Sobel y-taps are banded 128x128
matmuls on the tensor engine; the x-taps are folded into the same matmuls
via +-1-shifted rhs views of a W-padded prob slab (so conv zero-padding is
exact with no edge fixups).
"""

import numpy as np
from contextlib import ExitStack

import concourse.bass as bass
import concourse.bacc as bacc
import concourse.mybir as mybir
import concourse.tile as tile
from concourse import bass_utils
import concourse.dve_ops as dve_ops
from concourse.dve_spec import C0 as _C0, Spec as _Spec, Src0 as _Src0, \
    Src1 as _Src1, lower as _dve_lower, sq as _dve_sq
from concourse.dve_uop import DveOpSpec as _DveOpSpec
from operator import add as _op_add


def _register_custom(name, body, reference):
    for o in dve_ops.OPS:
        if o.name == name:
            return o
    spec = _Spec(body=body, accum=_op_add, accum_init=_C0, reference=reference)
    row = 1 + len(dve_ops.OPS)
    assert row < 0x20
    dve_ops._SUB_OPCODE_FOR_NAME[name] = row
    shas = {}
    for ver in ("v3", "v4"):
        try:
            uops = _dve_lower(spec, ver=ver)
            shas[ver] = _DveOpSpec(name=name, opcode=row, uops=uops,
                                   rd1_en=True).sha(ver)
        except Exception:
            pass
    op = dve_ops.DveOp(name, spec, subdim=False, uops_sha=shas)
    dve_ops.OPS.append(op)
    dve_ops.CUSTOM_DVE_SPECS[name] = spec
    return op


def _ref_sqsum(in0, in1, c0, c1, c2):
    b = (in0.astype(np.float32) ** 2 + in1.astype(np.float32) ** 2).astype(np.float32)
    return b, c0 + b.reshape(b.shape[0], -1).sum(axis=-1, keepdims=True)


def _ref_sqadd(in0, in1, c0, c1, c2):
    b = (in0.astype(np.float32) ** 2 + in1.astype(np.float32)).astype(np.float32)
    return b, c0 + b.reshape(b.shape[0], -1).sum(axis=-1, keepdims=True)


SQSUM = _register_custom("SQSUM_ANT", _dve_sq(_Src0) + _dve_sq(_Src1), _ref_sqsum)
SQADD = _register_custom("SQADD_ANT", _dve_sq(_Src0) + _Src1, _ref_sqadd)

F32 = mybir.dt.float32
BF16 = mybir.dt.bfloat16
NP_BF16 = mybir.dt.np(BF16)

# Problem constants (hardcoded per spec: nn_BoundaryDistillationLoss_87230785781774)
B, C, H, W = 4, 19, 512, 1024
NCORES = 8
ROWS_PER_CORE = (B * H) // NCORES          # 256
HIN = ROWS_PER_CORE + 2                    # 258 (one halo row each side)
# (in_row_start, n_in_rows, n_out_rows); out rows = in rows - 2 (valid conv)
SLABS = ((0, 128, 126), (126, 128, 126), (252, 6, 4))
EXP_CHUNK = 4                              # channels per exp/DMA chunk


def _band_weights(n=128):
    """lhsT matrices (already transposed for matmul) in bf16.

    sy[m] = p[m-1] + 2 p[m] + p[m+1]  -> A_s (symmetric)
    dy[m] = p[m+1] - p[m-1]           -> A_d ; lhsT = A_d.T
    """
    A_s = np.zeros((n, n), np.float32)
    A_d = np.zeros((n, n), np.float32)
    i = np.arange(n)
    A_s[i, i] = 2.0
    A_s[i[:-1], i[:-1] + 1] = 1.0
    A_s[i[1:], i[1:] - 1] = 1.0
    A_d[i[:-1], i[:-1] + 1] = 1.0
    A_d[i[1:], i[1:] - 1] = -1.0
    def shift(m):
        # out partition p computes conv row p+1, so DVE consumers start at
        # partition 0 (engines only support quadrant-aligned start partitions)
        t = m.T  # [k, out_row]
        s = np.zeros_like(t)
        s[:, : n - 1] = t[:, 1:]
        return s.astype(NP_BF16)

    w_sp = shift(A_s)                      # +A_s  (for +sy(w+1))
    w_sn = shift(-A_s)                     # -A_s  (for -sy(w-1))
    w_d = shift(A_d)                       # A_d   (w-1 and w+1 taps of gy)
    w_d2 = shift(2.0 * A_d)                # 2 A_d (center tap of gy)
    ident = np.eye(n, dtype=np.float32).astype(NP_BF16)
    return {"w_sp": w_sp, "w_sn": w_sn, "w_d": w_d, "w_d2": w_d2, "ident": ident}


def acc_layout(slabs, c, nwh=2):
    nq = len(slabs) * 2 * c * nwh          # (slab, tensor, chan, whalf)
    ns = len(slabs) * c                    # (slab, chan)
    return nq, ns


def build_nc(c_dim=C, w_dim=W, hin=HIN, slabs=SLABS):
    """Build the SPMD single-core Bass program."""
    assert w_dim % 512 == 0 or w_dim <= 512
    nwh = max(1, w_dim // 512)
    wchunk = w_dim // nwh
    nq, ns = acc_layout(slabs, c_dim, nwh)
    nacc = nq + ns

    nc = bacc.Bacc("TRN2", target_bir_lowering=False)
    xs = nc.dram_tensor("xs", [c_dim, hin, w_dim], F32, kind="ExternalInput")
    xt = nc.dram_tensor("xt", [c_dim, hin, w_dim], F32, kind="ExternalInput")
    wts = {
        name: nc.dram_tensor(name, [128, 128], BF16, kind="ExternalInput")
        for name in ("w_sp", "w_sn", "w_d", "w_d2", "ident")
    }
    acc_out = nc.dram_tensor("acc", [128, nacc], F32, kind="ExternalOutput")

    x_dram = (xs, xt)
    mult = mybir.AluOpType.mult
    add = mybir.AluOpType.add
    EXP = mybir.ActivationFunctionType.Exp
    SQRT = mybir.ActivationFunctionType.Sqrt
    SQUARE = mybir.ActivationFunctionType.Square

    with ExitStack() as ctx:
        tc = ctx.enter_context(tile.TileContext(nc))
        sb = ctx.enter_context(tc.tile_pool(name="sb", bufs=2))
        consts = ctx.enter_context(tc.tile_pool(name="consts", bufs=1))
        psum = ctx.enter_context(tc.tile_pool(name="psum", bufs=1, space="PSUM"))

        # constants
        w_sb = {}
        for name in wts:
            t = consts.tile([128, 128], BF16, tag=name)
            nc.sync.dma_start(out=t, in_=wts[name][:, :])
            w_sb[name] = t
        acc_sb = consts.tile([128, nacc], F32, tag="acc")
        nc.vector.memset(acc_sb[:, :], 0.0)

        # channel chunks for DMA+exp
        chunks = []
        c0 = 0
        while c0 < c_dim:
            cn = min(EXP_CHUNK, c_dim - c0)
            chunks.append((c0, cn))
            c0 += cn

        for si, (r0, nin, nout) in enumerate(slabs):
            pslabs = []
            for ti in range(2):
                ps = sb.tile([128, c_dim, w_dim + 4], BF16, tag=f"pslab{ti}", bufs=1)
                pslabs.append(ps)
                # zero the W-pad columns (1 and w_dim+2); data in [2, w_dim+2)
                # (2 pad cols keep per-channel rows 4B-aligned for DVE 2x mode)
                nc.vector.memset(ps[0:nin, :, 1:2], 0.0)
                nc.vector.memset(ps[0:nin, :, w_dim + 2 : w_dim + 3], 0.0)
                # exp into the padded slab (per channel: the per-instruction
                # sync-wait budget only allows waiting on ~1 producer)
                for cc in range(c_dim):
                    stg = sb.tile([128, w_dim], F32, tag="stage", bufs=4)
                    nc.sync.dma_start(
                        out=stg[0:nin, :],
                        in_=x_dram[ti][cc, r0 : r0 + nin, :],
                    )
                    nc.scalar.activation(
                        out=ps[0:nin, cc, 2 : 2 + w_dim],
                        in_=stg[0:nin, :],
                        func=EXP,
                    )
                # softmax denominator via identity-matmul accumulation
                r32 = sb.tile([128, w_dim], F32, tag="r32", bufs=2)
                for wh in range(nwh):
                    z = psum.tile([128, wchunk], F32, tag="z", bufs=2)
                    for cc in range(c_dim):
                        nc.tensor.matmul(
                            z[0:nin, :],
                            lhsT=w_sb["ident"][0:nin, 0:nin],
                            rhs=pslabs[ti][
                                0:nin, cc, 2 + wh * wchunk : 2 + (wh + 1) * wchunk
                            ],
                            start=(cc == 0),
                            stop=(cc == c_dim - 1),
                        )
                    nc.vector.reciprocal_approx_fast(
                        out=r32[0:nin, wh * wchunk : (wh + 1) * wchunk],
                        in_=z[0:nin, :],
                    )
                r16 = sb.tile([128, w_dim], BF16, tag="r16", bufs=2)
                nc.vector.tensor_copy(out=r16[0:nin, :], in_=r32[0:nin, :])
                for cc in range(c_dim):
                    nc.vector.tensor_mul(
                        out=pslabs[ti][0:nin, cc, 2 : 2 + w_dim],
                        in0=pslabs[ti][0:nin, cc, 2 : 2 + w_dim],
                        in1=r16[0:nin, 0:w_dim],
                    )

            # conv + squared-magnitude + cross term
            o = 0
            for cc in range(c_dim):
                q_tiles = []
                for ti in range(2):
                    g2 = sb.tile([128, w_dim], BF16, tag=f"g2_{ti}", bufs=2)
                    h2 = sb.tile([128, w_dim], BF16, tag=f"h2_{ti}", bufs=2)
                    q = sb.tile([128, w_dim], BF16, tag=f"q_{ti}", bufs=2)
                    for wh in range(nwh):
                        b0 = wh * wchunk
                        gx = psum.tile([128, wchunk], F32, tag="gx", bufs=3)
                        nc.tensor.matmul(
                            gx[:, :],
                            lhsT=w_sb["w_sp"][0:nin, :],
                            rhs=pslabs[ti][0:nin, cc, b0 + 3 : b0 + 3 + wchunk],
                            start=True,
                            stop=False,
                        )
                        nc.tensor.matmul(
                            gx[:, :],
                            lhsT=w_sb["w_sn"][0:nin, :],
                            rhs=pslabs[ti][0:nin, cc, b0 + 1 : b0 + 1 + wchunk],
                            start=False,
                            stop=True,
                        )
                        gy = psum.tile([128, wchunk], F32, tag="gy", bufs=3)
                        nc.tensor.matmul(
                            gy[:, :],
                            lhsT=w_sb["w_d"][0:nin, :],
                            rhs=pslabs[ti][0:nin, cc, b0 + 1 : b0 + 1 + wchunk],
                            start=True,
                            stop=False,
                        )
                        nc.tensor.matmul(
                            gy[:, :],
                            lhsT=w_sb["w_d2"][0:nin, :],
                            rhs=pslabs[ti][0:nin, cc, b0 + 2 : b0 + 2 + wchunk],
                            start=False,
                            stop=False,
                        )
                        nc.tensor.matmul(
                            gy[:, :],
                            lhsT=w_sb["w_d"][0:nin, :],
                            rhs=pslabs[ti][0:nin, cc, b0 + 3 : b0 + 3 + wchunk],
                            start=False,
                            stop=True,
                        )
                        qcol = ((si * 2 + ti) * c_dim + cc) * nwh + wh
                        acc_col = acc_sb[o : o + nout, qcol : qcol + 1]
                        if (cc + wh) % 4 == 0:
                            # X1: DVE copies gy out of PSUM, then one fused
                            # q = gx^2 + gy^2 (+running sum) custom op
                            nc.vector.tensor_copy(
                                out=h2[o : o + nout, b0 : b0 + wchunk],
                                in_=gy[o : o + nout, :],
                            )
                            nc.vector._custom_dve(
                                SQSUM,
                                out=q[o : o + nout, b0 : b0 + wchunk],
                                in0=gx[o : o + nout, :],
                                in1=h2[o : o + nout, b0 : b0 + wchunk],
                                s0=0.0,
                                accum_out=acc_col,
                            )
                        else:
                            # X2: ScalarE squares gx from PSUM, DVE fuses
                            # q = gy^2 + gx2 (+running sum)
                            nc.scalar.activation(
                                out=g2[o : o + nout, b0 : b0 + wchunk],
                                in_=gx[o : o + nout, :],
                                func=SQUARE,
                            )
                            nc.vector._custom_dve(
                                SQADD,
                                out=q[o : o + nout, b0 : b0 + wchunk],
                                in0=gy[o : o + nout, :],
                                in1=g2[o : o + nout, b0 : b0 + wchunk],
                                s0=0.0,
                                accum_out=acc_col,
                            )
                    q_tiles.append(q)
                p16 = sb.tile([128, w_dim], BF16, tag="p16", bufs=2)
                nc.vector.tensor_mul(
                    out=p16[o : o + nout, :],
                    in0=q_tiles[0][o : o + nout, :],
                    in1=q_tiles[1][o : o + nout, :],
                )
                psq = sb.tile([128, w_dim], BF16, tag="psq", bufs=2)
                scol = nq + si * c_dim + cc
                nc.scalar.activation(
                    out=psq[o : o + nout, :],
                    in_=p16[o : o + nout, :],
                    func=SQRT,
                    accum_out=acc_sb[o : o + nout, scol : scol + 1],
                )

        nc.sync.dma_start(out=acc_out[:, :], in_=acc_sb[:, :])
    if not nc.is_finalized():
        nc.finalize()
    return nc


def shard_inputs(student_logits, teacher_logits, c_dim=C, h_dim=H, w_dim=W,
                 ncores=NCORES):
    """Full (B,C,H,W) fp32 -> per-core dicts with (C, rows+2, W) halo shards."""
    b_dim = student_logits.shape[0]
    rows = (b_dim * h_dim) // ncores
    per_b = h_dim // rows if rows <= h_dim else 1
    in_maps = []
    wts = _band_weights()
    for k in range(ncores):
        g0 = k * rows
        bi, h0 = g0 // h_dim, g0 % h_dim
        m = {}
        for name, x in (("xs", student_logits), ("xt", teacher_logits)):
            img = x[bi]                                    # (C, H, W)
            sh = np.zeros((c_dim, rows + 2, w_dim), np.float32)
            lo, hi = h0 - 1, h0 + rows + 1
            src_lo, src_hi = max(lo, 0), min(hi, h_dim)
            sh[:, src_lo - lo : src_lo - lo + (src_hi - src_lo), :] = \
                np.asarray(img[:, src_lo:src_hi, :], np.float32)
            m[name] = sh
        for name, wv in wts.items():
            m[name] = wv
        in_maps.append(m)
    return in_maps


_NC_CACHE = {}


def _get_nc():
    key = "full"
    if key not in _NC_CACHE:
        _NC_CACHE[key] = build_nc()
    return _NC_CACHE[key]


def run_on_cores(in_maps, trace=False, **kw):
    nc = _get_nc()
    res = bass_utils.run_bass_kernel_spmd(
        nc, in_maps, core_ids=list(range(len(in_maps))), trace=trace, **kw
    )
    return res


def finish(results, slabs=SLABS, c_dim=C, n_total=None):
    nq, ns = acc_layout(slabs, c_dim)
    if n_total is None:
        n_total = B * C * H * W
    tq = 0.0
    tcross = 0.0
    for r in results:
        a = np.asarray(r["acc"], np.float64)
        tq += a[:, :nq].sum()
        tcross += a[:, nq:].sum()
    return np.float32((tq - 2.0 * tcross) / n_total)


def kernel(student_logits, teacher_logits):
    in_maps = shard_inputs(np.asarray(student_logits), np.asarray(teacher_logits))
    res = run_on_cores(in_maps)
    return finish(res.results)


# revision 11
# speedup vs baseline: 1.6760x; 1.6760x over previous
"""Trainium2 Bass kernel for nn_BoundaryDistillationLoss.

loss = mean((|grad(softmax(s))| - |grad(softmax(t))|)^2) with depthwise 3x3
Sobel gradients. Expanded as  [ sum(qs) + sum(qt) - 2*sum(sqrt(qs*qt)) ] / N
where q = gx^2 + gy^2, so no per-tensor sqrt is needed (one sqrt per pair).

Data parallel over B*H rows (2048) across 8 cores; each core gets a
(C, 258, W) halo-padded shard per tensor.  On-chip layout: h-rows on SBUF
partitions, (c, w) on the free dim.  The Sobel y-taps are banded 128x128
matmuls on the tensor engine; the x-taps are folded into the same matmuls
via +-1-shifted rhs views of a W-padded prob slab (so conv zero-padding is
exact with no edge fixups).  The 4-row tail that doesn't fit the 126-row
slab tiling is processed in a packed layout: partitions = (channel, row),
student/teacher side by side in the free dim, so it costs ~1/20 of a slab
instead of a full one.

Custom DVE ops (SQSUM/SQADD) fuse q = a^2 + b^2 with a running free-dim
sum; squares are split between ScalarE (Square activation reading PSUM)
and VectorE to balance engine load.
"""

import numpy as np
from contextlib import ExitStack

import concourse.bass as bass
import concourse.bacc as bacc
import concourse.mybir as mybir
import concourse.tile as tile
from concourse import bass_utils
import concourse.dve_ops as dve_ops
from concourse.dve_spec import C0 as _C0, Spec as _Spec, Src0 as _Src0, \
    Src1 as _Src1, lower as _dve_lower, sq as _dve_sq
from concourse.dve_uop import DveOpSpec as _DveOpSpec
from operator import add as _op_add


def _register_custom(name, body, reference):
    for o in dve_ops.OPS:
        if o.name == name:
            return o
    spec = _Spec(body=body, accum=_op_add, accum_init=_C0, reference=reference)
    row = 1 + len(dve_ops.OPS)
    assert row < 0x20
    dve_ops._SUB_OPCODE_FOR_NAME[name] = row
    shas = {}
    for ver in ("v3", "v4"):
        try:
            uops = _dve_lower(spec, ver=ver)
            shas[ver] = _DveOpSpec(name=name, opcode=row, uops=uops,
                                   rd1_en=True).sha(ver)
        except Exception:
            pass
    op = dve_ops.DveOp(name, spec, subdim=False, uops_sha=shas)
    dve_ops.OPS.append(op)
    dve_ops.CUSTOM_DVE_SPECS[name] = spec
    return op


def _ref_sqsum(in0, in1, c0, c1, c2):
    b = (in0.astype(np.float32) ** 2 + in1.astype(np.float32) ** 2).astype(np.float32)
    return b, c0 + b.reshape(b.shape[0], -1).sum(axis=-1, keepdims=True)


def _ref_sqadd(in0, in1, c0, c1, c2):
    b = (in0.astype(np.float32) ** 2 + in1.astype(np.float32)).astype(np.float32)
    return b, c0 + b.reshape(b.shape[0], -1).sum(axis=-1, keepdims=True)


SQSUM = _register_custom("SQSUM_ANT", _dve_sq(_Src0) + _dve_sq(_Src1), _ref_sqsum)
SQADD = _register_custom("SQADD_ANT", _dve_sq(_Src0) + _Src1, _ref_sqadd)

F32 = mybir.dt.float32
BF16 = mybir.dt.bfloat16
NP_BF16 = mybir.dt.np(BF16)

# Problem constants (hardcoded per spec: nn_BoundaryDistillationLoss_87230785781774)
B, C, H, W = 4, 19, 512, 1024
NCORES = 8
ROWS_PER_CORE = (B * H) // NCORES          # 256
HIN = ROWS_PER_CORE + 2                    # 258 (one halo row each side)
# main slabs: (in_row_start, n_in_rows, n_out_rows); out = in - 2 (valid conv)
MAIN_SLABS = ((0, 128, 126), (126, 128, 126))
REM = (252, 6)                             # packed tail: in rows 252..257 -> out 252..255
EXP_CHUNK = 4                              # channels per DMA+exp instruction


def _shifted_band(a, n, nfull=128):
    """lhsT [nfull, nfull] with lhsT[k, m] = a[m+1, k] (out row m = conv row
    m+1 so DVE consumers start at partition 0); a is [n, n]."""
    t = np.zeros((nfull, nfull), np.float32)
    t[:n, : n - 1] = a.T[:, 1:]
    return t


def _base_bands(n):
    A_s = np.zeros((n, n), np.float32)
    A_d = np.zeros((n, n), np.float32)
    i = np.arange(n)
    A_s[i, i] = 2.0
    A_s[i[:-1], i[:-1] + 1] = 1.0
    A_s[i[1:], i[1:] - 1] = 1.0
    A_d[i[:-1], i[:-1] + 1] = 1.0
    A_d[i[1:], i[1:] - 1] = -1.0
    return A_s, A_d


def _band_weights(c_dim=C, blk=6):
    A_s, A_d = _base_bands(128)
    out = {
        "w_sp": _shifted_band(A_s, 128),
        "w_sn": _shifted_band(-A_s, 128),
        "w_d": _shifted_band(A_d, 128),
        "w_d2": _shifted_band(2.0 * A_d, 128),
        "ident": np.eye(128, dtype=np.float32),
    }
    # packed-remainder block-diagonal bands: c_dim blocks of blk rows
    a_s, a_d = _base_bands(blk)
    npk = c_dim * blk
    assert npk <= 128
    for name, a in (("w_rsp", a_s), ("w_rsn", -a_s), ("w_rd", a_d),
                    ("w_rd2", 2.0 * a_d)):
        m = np.zeros((128, 128), np.float32)
        sb = _shifted_band(a, blk, blk)
        # out rows blk-2.. would be partial convs of the halo row; the
        # consumers read all packed partitions, so force them to zero
        sb[:, blk - 2 :] = 0.0
        for cblk in range(c_dim):
            m[cblk * blk : (cblk + 1) * blk, cblk * blk : (cblk + 1) * blk] = sb
        out[name] = m
    w_sel = np.zeros((128, 128), np.float32)   # z[i] = sum_c exp[c*blk+i]
    w_rep = np.zeros((128, 128), np.float32)   # rep[c*blk+i] = r[i]
    for cblk in range(c_dim):
        for i in range(blk):
            w_sel[cblk * blk + i, i] = 1.0
            w_rep[i, cblk * blk + i] = 1.0
    out["w_sel"] = w_sel
    out["w_rep"] = w_rep
    return {k: v.astype(NP_BF16) for k, v in out.items()}


def acc_layout(main_slabs, c, nwh=2, rem=True):
    nq = len(main_slabs) * 2 * c * nwh + (2 * nwh if rem else 0)
    ns = len(main_slabs) * c + (1 if rem else 0)
    return nq, ns


def build_nc(c_dim=C, w_dim=W, hin=HIN, main_slabs=MAIN_SLABS, rem=REM):
    nwh = max(1, w_dim // 512)
    wc = w_dim // nwh
    nq, ns = acc_layout(main_slabs, c_dim, nwh, rem is not None)
    nacc = nq + ns
    blk = rem[1] if rem is not None else 6
    npk = c_dim * blk

    nc = bacc.Bacc("TRN2", target_bir_lowering=False)
    xs = nc.dram_tensor("xs", [c_dim, hin, w_dim], F32, kind="ExternalInput")
    xt = nc.dram_tensor("xt", [c_dim, hin, w_dim], F32, kind="ExternalInput")
    wnames = ("w_sp", "w_sn", "w_d", "w_d2", "ident",
              "w_rsp", "w_rsn", "w_rd", "w_rd2", "w_sel", "w_rep")
    wts = {n: nc.dram_tensor(n, [128, 128], BF16, kind="ExternalInput")
           for n in wnames}
    acc_out = nc.dram_tensor("acc", [128, nacc], F32, kind="ExternalOutput")

    x_dram = (xs, xt)
    EXP = mybir.ActivationFunctionType.Exp
    SQRT = mybir.ActivationFunctionType.Sqrt
    SQUARE = mybir.ActivationFunctionType.Square

    qcol = iter(range(nq))
    scol = iter(range(nq, nacc))

    with ExitStack() as ctx:
        tc = ctx.enter_context(tile.TileContext(nc))
        sb = ctx.enter_context(tc.tile_pool(name="sb", bufs=2))
        consts = ctx.enter_context(tc.tile_pool(name="consts", bufs=1))
        psum = ctx.enter_context(tc.tile_pool(name="psum", bufs=1, space="PSUM"))

        w_sb = {}
        for name in wnames:
            t = consts.tile([128, 128], BF16, tag=name)
            nc.sync.dma_start(out=t, in_=wts[name][:, :])
            w_sb[name] = t
        acc_sb = consts.tile([128, nacc], F32, tag="acc")
        nc.vector.memset(acc_sb[:, :], 0.0)

        chunks = []
        c0 = 0
        while c0 < c_dim:
            cn = min(EXP_CHUNK, c_dim - c0)
            chunks.append((c0, cn))
            c0 += cn

        def squares(nout, cc, wh, gx, gy, q, g2, h2, b0):
            """q[0:nout, b0:b0+wc] = gx^2 + gy^2 (+ sum into a fresh acc col)."""
            col = next(qcol)
            acc_col = acc_sb[0:nout, col : col + 1]
            if (cc + wh) % 4 == 0:
                nc.vector.tensor_copy(out=h2[0:nout, b0 : b0 + wc],
                                      in_=gy[0:nout, :])
                nc.vector._custom_dve(
                    SQSUM, out=q[0:nout, b0 : b0 + wc], in0=gx[0:nout, :],
                    in1=h2[0:nout, b0 : b0 + wc], s0=0.0, accum_out=acc_col)
            else:
                nc.scalar.activation(out=g2[0:nout, b0 : b0 + wc],
                                     in_=gx[0:nout, :], func=SQUARE)
                nc.vector._custom_dve(
                    SQADD, out=q[0:nout, b0 : b0 + wc], in0=gy[0:nout, :],
                    in1=g2[0:nout, b0 : b0 + wc], s0=0.0, accum_out=acc_col)

        def conv_mms(wn_sp, wn_sn, wn_d, wn_d2, nin, ps_view, b0):
            """gx/gy psum tiles for one (c, T, wh); ps_view = [128, w+4] bf16."""
            gx = psum.tile([128, wc], F32, tag="gx", bufs=3)
            nc.tensor.matmul(gx[:, :], lhsT=w_sb[wn_sp][0:nin, :],
                             rhs=ps_view[0:nin, b0 + 3 : b0 + 3 + wc],
                             start=True, stop=False)
            nc.tensor.matmul(gx[:, :], lhsT=w_sb[wn_sn][0:nin, :],
                             rhs=ps_view[0:nin, b0 + 1 : b0 + 1 + wc],
                             start=False, stop=True)
            gy = psum.tile([128, wc], F32, tag="gy", bufs=3)
            nc.tensor.matmul(gy[:, :], lhsT=w_sb[wn_d][0:nin, :],
                             rhs=ps_view[0:nin, b0 + 1 : b0 + 1 + wc],
                             start=True, stop=False)
            nc.tensor.matmul(gy[:, :], lhsT=w_sb[wn_d2][0:nin, :],
                             rhs=ps_view[0:nin, b0 + 2 : b0 + 2 + wc],
                             start=False, stop=False)
            nc.tensor.matmul(gy[:, :], lhsT=w_sb[wn_d][0:nin, :],
                             rhs=ps_view[0:nin, b0 + 3 : b0 + 3 + wc],
                             start=False, stop=True)
            return gx, gy

        # ---------------- main slabs ----------------
        for si, (r0, nin, nout) in enumerate(main_slabs):
            pslabs = []
            for ti in range(2):
                ps = sb.tile([128, c_dim, w_dim + 4], BF16, tag=f"pslab{ti}",
                             bufs=1)
                pslabs.append(ps)
                nc.vector.memset(ps[0:nin, :, 1:2], 0.0)
                nc.vector.memset(ps[0:nin, :, w_dim + 2 : w_dim + 3], 0.0)
                for (cc0, cn) in chunks:
                    stg = sb.tile([128, cn, w_dim], F32, tag="stage", bufs=3)
                    nc.sync.dma_start(
                        out=stg[0:nin, :, :],
                        in_=x_dram[ti][cc0 : cc0 + cn, r0 : r0 + nin, :]
                        .rearrange("c h w -> h c w"))
                    nc.scalar.activation(
                        out=ps[0:nin, cc0 : cc0 + cn, 2 : 2 + w_dim],
                        in_=stg[0:nin, :, :], func=EXP)
                r32 = sb.tile([128, w_dim], F32, tag="r32", bufs=2)
                for wh in range(nwh):
                    z = psum.tile([128, wc], F32, tag="z", bufs=2)
                    for cc in range(c_dim):
                        nc.tensor.matmul(
                            z[0:nin, :], lhsT=w_sb["ident"][0:nin, 0:nin],
                            rhs=ps[0:nin, cc, 2 + wh * wc : 2 + (wh + 1) * wc],
                            start=(cc == 0), stop=(cc == c_dim - 1))
                    nc.vector.reciprocal_approx_fast(
                        out=r32[0:nin, wh * wc : (wh + 1) * wc], in_=z[0:nin, :])
                r16 = sb.tile([128, w_dim], BF16, tag="r16", bufs=2)
                nc.vector.tensor_copy(out=r16[0:nin, :], in_=r32[0:nin, :])
                for cc in range(c_dim):
                    nc.vector.tensor_mul(
                        out=ps[0:nin, cc, 2 : 2 + w_dim],
                        in0=ps[0:nin, cc, 2 : 2 + w_dim],
                        in1=r16[0:nin, 0:w_dim])

            for cc in range(c_dim):
                q_tiles = []
                for ti in range(2):
                    g2 = sb.tile([128, w_dim], BF16, tag=f"g2_{ti}", bufs=2)
                    h2 = sb.tile([128, w_dim], BF16, tag=f"h2_{ti}", bufs=2)
                    q = sb.tile([128, w_dim], BF16, tag=f"q_{ti}", bufs=2)
                    for wh in range(nwh):
                        gx, gy = conv_mms("w_sp", "w_sn", "w_d", "w_d2", nin,
                                          pslabs[ti][:, cc, :], wh * wc)
                        squares(nout, cc, wh, gx, gy, q, g2, h2, wh * wc)
                    q_tiles.append(q)
                p16 = sb.tile([128, w_dim], BF16, tag="p16", bufs=2)
                nc.vector.tensor_mul(out=p16[0:nout, :],
                                     in0=q_tiles[0][0:nout, :],
                                     in1=q_tiles[1][0:nout, :])
                psq = sb.tile([128, w_dim], BF16, tag="psq", bufs=2)
                col = next(scol)
                nc.scalar.activation(out=psq[0:nout, :], in_=p16[0:nout, :],
                                     func=SQRT,
                                     accum_out=acc_sb[0:nout, col : col + 1])

        # ---------------- packed remainder ----------------
        if rem is not None:
            r0 = rem[0]
            rps = sb.tile([128, 2, w_dim + 4], BF16, tag="rem_ps", bufs=1)
            nc.vector.memset(rps[0:npk, :, 1:2], 0.0)
            nc.vector.memset(rps[0:npk, :, w_dim + 2 : w_dim + 3], 0.0)
            stg = sb.tile([128, 2, w_dim], F32, tag="rem_stage", bufs=1)
            for ti in range(2):
                for cc in range(c_dim):
                    nc.sync.dma_start(
                        out=stg[cc * blk : (cc + 1) * blk, ti, :],
                        in_=x_dram[ti][cc, r0 : r0 + blk, :])
            nc.scalar.activation(out=rps[0:npk, :, 2 : 2 + w_dim],
                                 in_=stg[0:npk, :, :], func=EXP)
            for ti in range(2):
                r32 = sb.tile([128, w_dim], F32, tag="r32", bufs=2)
                for wh in range(nwh):
                    z = psum.tile([128, wc], F32, tag="z", bufs=2)
                    nc.tensor.matmul(
                        z[0:blk, :], lhsT=w_sb["w_sel"][0:npk, 0:blk],
                        rhs=rps[0:npk, ti, 2 + wh * wc : 2 + (wh + 1) * wc],
                        start=True, stop=True)
                    nc.vector.reciprocal_approx_fast(
                        out=r32[0:blk, wh * wc : (wh + 1) * wc], in_=z[0:blk, :])
                r16 = sb.tile([128, w_dim], BF16, tag="r16", bufs=2)
                nc.vector.tensor_copy(out=r16[0:blk, :], in_=r32[0:blk, :])
                for wh in range(nwh):
                    rrep = psum.tile([128, wc], F32, tag="gx", bufs=3)
                    nc.tensor.matmul(
                        rrep[0:npk, :], lhsT=w_sb["w_rep"][0:blk, 0:npk],
                        rhs=r16[0:blk, wh * wc : (wh + 1) * wc],
                        start=True, stop=True)
                    nc.vector.tensor_mul(
                        out=rps[0:npk, ti, 2 + wh * wc : 2 + (wh + 1) * wc],
                        in0=rps[0:npk, ti, 2 + wh * wc : 2 + (wh + 1) * wc],
                        in1=rrep[0:npk, :])
            q_tiles = []
            for ti in range(2):
                g2 = sb.tile([128, w_dim], BF16, tag="g2_0", bufs=2)
                h2 = sb.tile([128, w_dim], BF16, tag="h2_0", bufs=2)
                q = sb.tile([128, w_dim], BF16, tag=f"q_{ti}", bufs=2)
                for wh in range(nwh):
                    gx, gy = conv_mms("w_rsp", "w_rsn", "w_rd", "w_rd2", npk,
                                      rps[:, ti, :], wh * wc)
                    squares(npk, 1 + wh, wh, gx, gy, q, g2, h2, wh * wc)
                q_tiles.append(q)
            p16 = sb.tile([128, w_dim], BF16, tag="p16", bufs=2)
            nc.vector.tensor_mul(out=p16[0:npk, :], in0=q_tiles[0][0:npk, :],
                                 in1=q_tiles[1][0:npk, :])
            psq = sb.tile([128, w_dim], BF16, tag="psq", bufs=2)
            col = next(scol)
            nc.scalar.activation(out=psq[0:npk, :], in_=p16[0:npk, :],
                                 func=SQRT,
                                 accum_out=acc_sb[0:npk, col : col + 1])

        nc.sync.dma_start(out=acc_out[:, :], in_=acc_sb[:, :])
    if not nc.is_finalized():
        nc.finalize()
    return nc


def shard_inputs(student_logits, teacher_logits, c_dim=C, h_dim=H, w_dim=W,
                 ncores=NCORES):
    """Full (B,C,H,W) fp32 -> per-core dicts with (C, rows+2, W) halo shards."""
    b_dim = student_logits.shape[0]
    rows = (b_dim * h_dim) // ncores
    in_maps = []
    wts = _band_weights()
    for k in range(ncores):
        g0 = k * rows
        bi, h0 = g0 // h_dim, g0 % h_dim
        m = {}
        for name, x in (("xs", student_logits), ("xt", teacher_logits)):
            img = x[bi]                                    # (C, H, W)
            sh = np.zeros((c_dim, rows + 2, w_dim), np.float32)
            lo, hi = h0 - 1, h0 + rows + 1
            src_lo, src_hi = max(lo, 0), min(hi, h_dim)
            sh[:, src_lo - lo : src_lo - lo + (src_hi - src_lo), :] = \
                np.asarray(img[:, src_lo:src_hi, :], np.float32)
            m[name] = sh
        for name, wv in wts.items():
            m[name] = wv
        in_maps.append(m)
    return in_maps


_NC_CACHE = {}


def _get_nc():
    key = "full"
    if key not in _NC_CACHE:
        _NC_CACHE[key] = build_nc()
    return _NC_CACHE[key]


def run_on_cores(in_maps, trace=False, **kw):
    nc = _get_nc()
    res = bass_utils.run_bass_kernel_spmd(
        nc, in_maps, core_ids=list(range(len(in_maps))), trace=trace, **kw
    )
    return res


def finish(results, main_slabs=MAIN_SLABS, c_dim=C, n_total=None, nwh=2,
           rem=True):
    nq, ns = acc_layout(main_slabs, c_dim, nwh, rem)
    if n_total is None:
        n_total = B * C * H * W
    tq = 0.0
    tcross = 0.0
    for r in results:
        a = np.asarray(r["acc"], np.float64)
        tq += a[:, :nq].sum()
        tcross += a[:, nq:].sum()
    return np.float32((tq - 2.0 * tcross) / n_total)


def kernel(student_logits, teacher_logits):
    in_maps = shard_inputs(np.asarray(student_logits), np.asarray(teacher_logits))
    res = run_on_cores(in_maps)
    return finish(res.results)


# revision 12
# speedup vs baseline: 1.6825x; 1.0039x over previous
"""Trainium2 Bass kernel for nn_BoundaryDistillationLoss.

loss = mean((|grad(softmax(s))| - |grad(softmax(t))|)^2) with depthwise 3x3
Sobel gradients. Expanded as  [ sum(qs) + sum(qt) - 2*sum(sqrt(qs*qt)) ] / N
where q = gx^2 + gy^2, so no per-tensor sqrt is needed (one sqrt per pair).

Data parallel over B*H rows (2048) across 8 cores; each core gets a
(C, 258, W) halo-padded shard per tensor.  On-chip layout: h-rows on SBUF
partitions, (c, w) on the free dim.  The Sobel y-taps are banded 128x128
matmuls on the tensor engine; the x-taps are folded into the same matmuls
via +-1-shifted rhs views of a W-padded prob slab (so conv zero-padding is
exact with no edge fixups).  The 4-row tail that doesn't fit the 126-row
slab tiling is processed in a packed layout: partitions = (channel, row),
student/teacher side by side in the free dim, so it costs ~1/20 of a slab
instead of a full one.

Custom DVE ops (SQSUM/SQADD) fuse q = a^2 + b^2 with a running free-dim
sum; squares are split between ScalarE (Square activation reading PSUM)
and VectorE to balance engine load.
"""

import numpy as np
from contextlib import ExitStack

import concourse.bass as bass
import concourse.bacc as bacc
import concourse.mybir as mybir
import concourse.tile as tile
from concourse import bass_utils
import concourse.dve_ops as dve_ops
from concourse.dve_spec import C0 as _C0, Spec as _Spec, Src0 as _Src0, \
    Src1 as _Src1, lower as _dve_lower, sq as _dve_sq
from concourse.dve_uop import DveOpSpec as _DveOpSpec
from operator import add as _op_add


def _register_custom(name, body, reference):
    for o in dve_ops.OPS:
        if o.name == name:
            return o
    spec = _Spec(body=body, accum=_op_add, accum_init=_C0, reference=reference)
    row = 1 + len(dve_ops.OPS)
    assert row < 0x20
    dve_ops._SUB_OPCODE_FOR_NAME[name] = row
    shas = {}
    for ver in ("v3", "v4"):
        try:
            uops = _dve_lower(spec, ver=ver)
            shas[ver] = _DveOpSpec(name=name, opcode=row, uops=uops,
                                   rd1_en=True).sha(ver)
        except Exception:
            pass
    op = dve_ops.DveOp(name, spec, subdim=False, uops_sha=shas)
    dve_ops.OPS.append(op)
    dve_ops.CUSTOM_DVE_SPECS[name] = spec
    return op


def _ref_sqsum(in0, in1, c0, c1, c2):
    b = (in0.astype(np.float32) ** 2 + in1.astype(np.float32) ** 2).astype(np.float32)
    return b, c0 + b.reshape(b.shape[0], -1).sum(axis=-1, keepdims=True)


def _ref_sqadd(in0, in1, c0, c1, c2):
    b = (in0.astype(np.float32) ** 2 + in1.astype(np.float32)).astype(np.float32)
    return b, c0 + b.reshape(b.shape[0], -1).sum(axis=-1, keepdims=True)


SQSUM = _register_custom("SQSUM_ANT", _dve_sq(_Src0) + _dve_sq(_Src1), _ref_sqsum)
SQADD = _register_custom("SQADD_ANT", _dve_sq(_Src0) + _Src1, _ref_sqadd)

F32 = mybir.dt.float32
BF16 = mybir.dt.bfloat16
NP_BF16 = mybir.dt.np(BF16)

# Problem constants (hardcoded per spec: nn_BoundaryDistillationLoss_87230785781774)
B, C, H, W = 4, 19, 512, 1024
NCORES = 8
ROWS_PER_CORE = (B * H) // NCORES          # 256
HIN = ROWS_PER_CORE + 2                    # 258 (one halo row each side)
# main slabs: (in_row_start, n_in_rows, n_out_rows); out = in - 2 (valid conv)
MAIN_SLABS = ((0, 128, 126), (126, 128, 126))
REM = (252, 6)                             # packed tail: in rows 252..257 -> out 252..255
EXP_CHUNK = 4                              # channels per DMA+exp instruction


def _shifted_band(a, n, nfull=128):
    """lhsT [nfull, nfull] with lhsT[k, m] = a[m+1, k] (out row m = conv row
    m+1 so DVE consumers start at partition 0); a is [n, n]."""
    t = np.zeros((nfull, nfull), np.float32)
    t[:n, : n - 1] = a.T[:, 1:]
    return t


def _base_bands(n):
    A_s = np.zeros((n, n), np.float32)
    A_d = np.zeros((n, n), np.float32)
    i = np.arange(n)
    A_s[i, i] = 2.0
    A_s[i[:-1], i[:-1] + 1] = 1.0
    A_s[i[1:], i[1:] - 1] = 1.0
    A_d[i[:-1], i[:-1] + 1] = 1.0
    A_d[i[1:], i[1:] - 1] = -1.0
    return A_s, A_d


def _band_weights(c_dim=C, blk=6):
    A_s, A_d = _base_bands(128)
    out = {
        "w_sp": _shifted_band(A_s, 128),
        "w_sn": _shifted_band(-A_s, 128),
        "w_d": _shifted_band(A_d, 128),
        "w_d2": _shifted_band(2.0 * A_d, 128),
        "ident": np.eye(128, dtype=np.float32),
    }
    # packed-remainder block-diagonal bands: c_dim blocks of blk rows
    a_s, a_d = _base_bands(blk)
    npk = c_dim * blk
    assert npk <= 128
    for name, a in (("w_rsp", a_s), ("w_rsn", -a_s), ("w_rd", a_d),
                    ("w_rd2", 2.0 * a_d)):
        m = np.zeros((128, 128), np.float32)
        sb = _shifted_band(a, blk, blk)
        # out rows blk-2.. would be partial convs of the halo row; the
        # consumers read all packed partitions, so force them to zero
        sb[:, blk - 2 :] = 0.0
        for cblk in range(c_dim):
            m[cblk * blk : (cblk + 1) * blk, cblk * blk : (cblk + 1) * blk] = sb
        out[name] = m
    w_sel = np.zeros((128, 128), np.float32)   # z[i] = sum_c exp[c*blk+i]
    w_rep = np.zeros((128, 128), np.float32)   # rep[c*blk+i] = r[i]
    for cblk in range(c_dim):
        for i in range(blk):
            w_sel[cblk * blk + i, i] = 1.0
            w_rep[i, cblk * blk + i] = 1.0
    out["w_sel"] = w_sel
    out["w_rep"] = w_rep
    return {k: v.astype(NP_BF16) for k, v in out.items()}


def acc_layout(main_slabs, c, nwh=2, rem=True):
    nq = len(main_slabs) * 2 * c * nwh + (2 * nwh if rem else 0)
    ns = len(main_slabs) * ((c + 1) // 2) + (1 if rem else 0)
    return nq, ns


def build_nc(c_dim=C, w_dim=W, hin=HIN, main_slabs=MAIN_SLABS, rem=REM):
    nwh = max(1, w_dim // 512)
    wc = w_dim // nwh
    nq, ns = acc_layout(main_slabs, c_dim, nwh, rem is not None)
    nacc = nq + ns
    blk = rem[1] if rem is not None else 6
    npk = c_dim * blk

    nc = bacc.Bacc("TRN2", target_bir_lowering=False)
    xs = nc.dram_tensor("xs", [c_dim, hin, w_dim], F32, kind="ExternalInput")
    xt = nc.dram_tensor("xt", [c_dim, hin, w_dim], F32, kind="ExternalInput")
    wnames = ("w_sp", "w_sn", "w_d", "w_d2", "ident",
              "w_rsp", "w_rsn", "w_rd", "w_rd2", "w_sel", "w_rep")
    wts = {n: nc.dram_tensor(n, [128, 128], BF16, kind="ExternalInput")
           for n in wnames}
    acc_out = nc.dram_tensor("acc", [128, nacc], F32, kind="ExternalOutput")

    x_dram = (xs, xt)
    EXP = mybir.ActivationFunctionType.Exp
    SQRT = mybir.ActivationFunctionType.Sqrt
    SQUARE = mybir.ActivationFunctionType.Square

    qcol = iter(range(nq))
    scol = iter(range(nq, nacc))

    with ExitStack() as ctx:
        tc = ctx.enter_context(tile.TileContext(nc))
        sb = ctx.enter_context(tc.tile_pool(name="sb", bufs=2))
        consts = ctx.enter_context(tc.tile_pool(name="consts", bufs=1))
        psum = ctx.enter_context(tc.tile_pool(name="psum", bufs=1, space="PSUM"))

        w_sb = {}
        for name in wnames:
            t = consts.tile([128, 128], BF16, tag=name)
            nc.sync.dma_start(out=t, in_=wts[name][:, :])
            w_sb[name] = t
        acc_sb = consts.tile([128, nacc], F32, tag="acc")
        nc.vector.memset(acc_sb[:, :], 0.0)

        chunks = []
        c0 = 0
        while c0 < c_dim:
            cn = min(EXP_CHUNK, c_dim - c0)
            chunks.append((c0, cn))
            c0 += cn

        def squares(nout, cc, wh, gx, gy, q, g2, h2, b0):
            """q[0:nout, b0:b0+wc] = gx^2 + gy^2 (+ sum into a fresh acc col)."""
            col = next(qcol)
            acc_col = acc_sb[0:nout, col : col + 1]
            if (cc + wh) % 3 == 0:
                nc.vector.tensor_copy(out=h2[0:nout, b0 : b0 + wc],
                                      in_=gy[0:nout, :])
                nc.vector._custom_dve(
                    SQSUM, out=q[0:nout, b0 : b0 + wc], in0=gx[0:nout, :],
                    in1=h2[0:nout, b0 : b0 + wc], s0=0.0, accum_out=acc_col)
            else:
                nc.scalar.activation(out=g2[0:nout, b0 : b0 + wc],
                                     in_=gx[0:nout, :], func=SQUARE)
                nc.vector._custom_dve(
                    SQADD, out=q[0:nout, b0 : b0 + wc], in0=gy[0:nout, :],
                    in1=g2[0:nout, b0 : b0 + wc], s0=0.0, accum_out=acc_col)

        def conv_mms(wn_sp, wn_sn, wn_d, wn_d2, nin, ps_view, b0):
            """gx/gy psum tiles for one (c, T, wh); ps_view = [128, w+4] bf16."""
            gx = psum.tile([128, wc], F32, tag="gx", bufs=3)
            nc.tensor.matmul(gx[:, :], lhsT=w_sb[wn_sp][0:nin, :],
                             rhs=ps_view[0:nin, b0 + 3 : b0 + 3 + wc],
                             start=True, stop=False)
            nc.tensor.matmul(gx[:, :], lhsT=w_sb[wn_sn][0:nin, :],
                             rhs=ps_view[0:nin, b0 + 1 : b0 + 1 + wc],
                             start=False, stop=True)
            gy = psum.tile([128, wc], F32, tag="gy", bufs=3)
            nc.tensor.matmul(gy[:, :], lhsT=w_sb[wn_d][0:nin, :],
                             rhs=ps_view[0:nin, b0 + 1 : b0 + 1 + wc],
                             start=True, stop=False)
            nc.tensor.matmul(gy[:, :], lhsT=w_sb[wn_d2][0:nin, :],
                             rhs=ps_view[0:nin, b0 + 2 : b0 + 2 + wc],
                             start=False, stop=False)
            nc.tensor.matmul(gy[:, :], lhsT=w_sb[wn_d][0:nin, :],
                             rhs=ps_view[0:nin, b0 + 3 : b0 + 3 + wc],
                             start=False, stop=True)
            return gx, gy

        # ---------------- main slabs ----------------
        for si, (r0, nin, nout) in enumerate(main_slabs):
            pslabs = []
            for ti in range(2):
                ps = sb.tile([128, c_dim, w_dim + 4], BF16, tag=f"pslab{ti}",
                             bufs=1)
                pslabs.append(ps)
                nc.vector.memset(
                    ps[0:nin, :, 1 : w_dim + 3 : w_dim + 1], 0.0)
                for (cc0, cn) in chunks:
                    stg = sb.tile([128, cn, w_dim], F32, tag="stage", bufs=3)
                    nc.sync.dma_start(
                        out=stg[0:nin, :, :],
                        in_=x_dram[ti][cc0 : cc0 + cn, r0 : r0 + nin, :]
                        .rearrange("c h w -> h c w"))
                    nc.scalar.activation(
                        out=ps[0:nin, cc0 : cc0 + cn, 2 : 2 + w_dim],
                        in_=stg[0:nin, :, :], func=EXP)
                r32 = sb.tile([128, w_dim], F32, tag="r32", bufs=2)
                for wh in range(nwh):
                    z = psum.tile([128, wc], F32, tag="z", bufs=2)
                    for cc in range(c_dim):
                        nc.tensor.matmul(
                            z[0:nin, :], lhsT=w_sb["ident"][0:nin, 0:nin],
                            rhs=ps[0:nin, cc, 2 + wh * wc : 2 + (wh + 1) * wc],
                            start=(cc == 0), stop=(cc == c_dim - 1))
                    nc.vector.reciprocal_approx_fast(
                        out=r32[0:nin, wh * wc : (wh + 1) * wc], in_=z[0:nin, :])
                r16 = sb.tile([128, w_dim], BF16, tag="r16", bufs=2)
                nc.vector.tensor_copy(out=r16[0:nin, :], in_=r32[0:nin, :])
                for cc in range(c_dim):
                    nc.vector.tensor_mul(
                        out=ps[0:nin, cc, 2 : 2 + w_dim],
                        in0=ps[0:nin, cc, 2 : 2 + w_dim],
                        in1=r16[0:nin, 0:w_dim])

            p16 = None
            for cc in range(c_dim):
                q_tiles = []
                for ti in range(2):
                    g2 = sb.tile([128, w_dim], BF16, tag=f"g2_{ti}", bufs=2)
                    h2 = sb.tile([128, w_dim], BF16, tag=f"h2_{ti}", bufs=2)
                    q = sb.tile([128, w_dim], BF16, tag=f"q_{ti}", bufs=2)
                    for wh in range(nwh):
                        gx, gy = conv_mms("w_sp", "w_sn", "w_d", "w_d2", nin,
                                          pslabs[ti][:, cc, :], wh * wc)
                        squares(nout, cc, wh, gx, gy, q, g2, h2, wh * wc)
                    q_tiles.append(q)
                if cc % 2 == 0:
                    p16 = sb.tile([128, 2, w_dim], BF16, tag="p16", bufs=2)
                nc.gpsimd.tensor_mul(out=p16[0:nout, cc % 2, :],
                                     in0=q_tiles[0][0:nout, :],
                                     in1=q_tiles[1][0:nout, :])
                if cc % 2 == 1 or cc == c_dim - 1:
                    nsl = cc % 2 + 1
                    psq = sb.tile([128, 2, w_dim], BF16, tag="psq", bufs=2)
                    col = next(scol)
                    nc.scalar.activation(
                        out=psq[0:nout, 0:nsl, :],
                        in_=p16[0:nout, 0:nsl, :], func=SQRT,
                        accum_out=acc_sb[0:nout, col : col + 1])

        # ---------------- packed remainder ----------------
        if rem is not None:
            r0 = rem[0]
            rps = sb.tile([128, 2, w_dim + 4], BF16, tag="rem_ps", bufs=1)
            nc.vector.memset(rps[0:npk, :, 1 : w_dim + 3 : w_dim + 1], 0.0)
            stg = sb.tile([128, 2, w_dim], F32, tag="rem_stage", bufs=1)
            for ti in range(2):
                for cc in range(c_dim):
                    nc.sync.dma_start(
                        out=stg[cc * blk : (cc + 1) * blk, ti, :],
                        in_=x_dram[ti][cc, r0 : r0 + blk, :])
            nc.scalar.activation(out=rps[0:npk, :, 2 : 2 + w_dim],
                                 in_=stg[0:npk, :, :], func=EXP)
            for ti in range(2):
                r32 = sb.tile([128, w_dim], F32, tag="r32", bufs=2)
                for wh in range(nwh):
                    z = psum.tile([128, wc], F32, tag="z", bufs=2)
                    nc.tensor.matmul(
                        z[0:blk, :], lhsT=w_sb["w_sel"][0:npk, 0:blk],
                        rhs=rps[0:npk, ti, 2 + wh * wc : 2 + (wh + 1) * wc],
                        start=True, stop=True)
                    nc.vector.reciprocal_approx_fast(
                        out=r32[0:blk, wh * wc : (wh + 1) * wc], in_=z[0:blk, :])
                r16 = sb.tile([128, w_dim], BF16, tag="r16", bufs=2)
                nc.vector.tensor_copy(out=r16[0:blk, :], in_=r32[0:blk, :])
                for wh in range(nwh):
                    rrep = psum.tile([128, wc], F32, tag="gx", bufs=3)
                    nc.tensor.matmul(
                        rrep[0:npk, :], lhsT=w_sb["w_rep"][0:blk, 0:npk],
                        rhs=r16[0:blk, wh * wc : (wh + 1) * wc],
                        start=True, stop=True)
                    nc.vector.tensor_mul(
                        out=rps[0:npk, ti, 2 + wh * wc : 2 + (wh + 1) * wc],
                        in0=rps[0:npk, ti, 2 + wh * wc : 2 + (wh + 1) * wc],
                        in1=rrep[0:npk, :])
            q_tiles = []
            for ti in range(2):
                g2 = sb.tile([128, w_dim], BF16, tag="g2_0", bufs=2)
                h2 = sb.tile([128, w_dim], BF16, tag="h2_0", bufs=2)
                q = sb.tile([128, w_dim], BF16, tag=f"q_{ti}", bufs=2)
                for wh in range(nwh):
                    gx, gy = conv_mms("w_rsp", "w_rsn", "w_rd", "w_rd2", npk,
                                      rps[:, ti, :], wh * wc)
                    squares(npk, 1 + wh, wh, gx, gy, q, g2, h2, wh * wc)
                q_tiles.append(q)
            p16 = sb.tile([128, 2, w_dim], BF16, tag="p16", bufs=2)
            nc.gpsimd.tensor_mul(out=p16[0:npk, 0, :], in0=q_tiles[0][0:npk, :],
                                 in1=q_tiles[1][0:npk, :])
            psq = sb.tile([128, 2, w_dim], BF16, tag="psq", bufs=2)
            col = next(scol)
            nc.scalar.activation(out=psq[0:npk, 0, :], in_=p16[0:npk, 0, :],
                                 func=SQRT,
                                 accum_out=acc_sb[0:npk, col : col + 1])

        nc.sync.dma_start(out=acc_out[:, :], in_=acc_sb[:, :])
    if not nc.is_finalized():
        nc.finalize()
    return nc


def shard_inputs(student_logits, teacher_logits, c_dim=C, h_dim=H, w_dim=W,
                 ncores=NCORES):
    """Full (B,C,H,W) fp32 -> per-core dicts with (C, rows+2, W) halo shards."""
    b_dim = student_logits.shape[0]
    rows = (b_dim * h_dim) // ncores
    in_maps = []
    wts = _band_weights()
    for k in range(ncores):
        g0 = k * rows
        bi, h0 = g0 // h_dim, g0 % h_dim
        m = {}
        for name, x in (("xs", student_logits), ("xt", teacher_logits)):
            img = x[bi]                                    # (C, H, W)
            sh = np.zeros((c_dim, rows + 2, w_dim), np.float32)
            lo, hi = h0 - 1, h0 + rows + 1
            src_lo, src_hi = max(lo, 0), min(hi, h_dim)
            sh[:, src_lo - lo : src_lo - lo + (src_hi - src_lo), :] = \
                np.asarray(img[:, src_lo:src_hi, :], np.float32)
            m[name] = sh
        for name, wv in wts.items():
            m[name] = wv
        in_maps.append(m)
    return in_maps


_NC_CACHE = {}


def _get_nc():
    key = "full"
    if key not in _NC_CACHE:
        _NC_CACHE[key] = build_nc()
    return _NC_CACHE[key]


def run_on_cores(in_maps, trace=False, **kw):
    nc = _get_nc()
    res = bass_utils.run_bass_kernel_spmd(
        nc, in_maps, core_ids=list(range(len(in_maps))), trace=trace, **kw
    )
    return res


def finish(results, main_slabs=MAIN_SLABS, c_dim=C, n_total=None, nwh=2,
           rem=True):
    nq, ns = acc_layout(main_slabs, c_dim, nwh, rem)
    if n_total is None:
        n_total = B * C * H * W
    tq = 0.0
    tcross = 0.0
    for r in results:
        a = np.asarray(r["acc"], np.float64)
        tq += a[:, :nq].sum()
        tcross += a[:, nq:].sum()
    return np.float32((tq - 2.0 * tcross) / n_total)


def kernel(student_logits, teacher_logits):
    in_maps = shard_inputs(np.asarray(student_logits), np.asarray(teacher_logits))
    res = run_on_cores(in_maps)
    return finish(res.results)
